# revision 17
# baseline (speedup 1.0000x reference)
"""Distributed Trainium2 Bass kernel for nn_Attention_62766652063769 (v6).

Reference computation (B=4, T=2048, C=1024, H=16, HD=64):
    qkv = x @ W_qkv^T ; split into q, k, v heads
    q, k <- RoPE(q), RoPE(k)   (interleaved-pair rotation)
    attn = softmax(q k^T / sqrt(HD))   (mask is all-ones -> no masking)
    out  = (attn @ v) @ W_proj^T

Sharding (tensor-parallel over heads, per the sharding hint): core
c = 2*b + hh owns batch b and head-half hh (8 of 16 heads), over the
FULL 2048 query tokens.  Q/K/V projections and attention are computed
only for the core's own heads (no redundant K/V work); the output
projection is row-sharded over the core's 512 att channels, producing
a bf16 PARTIAL result per core which the host sums per batch at
gather time (out[b] = partial[2b] + partial[2b+1]).

Device structure per core (4 local head pairs x 2 query-half
sub-sweeps of 1024 q tokens, identical inner loop each):
  - Merged score tiles: one PSUM tile [128, 1024] per (kt, q-chunk)
    holds BOTH heads side by side via tile_position (0,0)/(64,0) ->
    row-tiled concurrent matmuls, exp starts after 2 MMs.
  - ACT runs ONLY Exp (one table load).  Softmax denominator
    reciprocal is computed on a compact [128, 16] reshape (DRAM
    round-trip) so the slow DVE reciprocal touches 16 elems/lane.
  - OT (attn @ V) deferred 2 kt-tiles, A,A,B,B stationary reuse,
    65th V column accumulates the softmax denominator.
  - Filler grants (next Q/K/V projections + rope) are interleaved
    into the ACT-bound attention loop in PAIRS (grant PSUM tiles
    share the score ring; odd counts shift its recycle phase).
  - Proj phase: per 128-token tile, head/tail split so the last
    pair's late-normalized att never blocks the in-order PE queue;
    bf16 output halves DMA'd on sync+scalar queues.
"""

import os
import re
import sys
import types

if "/opt/trn_rl_repo" not in sys.path:
    sys.path.insert(0, "/opt/trn_rl_repo")

import ml_dtypes
import numpy as np

import bass_rust
import concourse.bass as bass
import concourse.mybir as mybir
from concourse import bass_utils
from concourse.tile import TileContext, ScopedClock

# ---------------------------------------------------------------------------
# Environment patches (same as v1/v2)
# ---------------------------------------------------------------------------

def _patched_drain_and_barrier(self, tick_clock, wait_clock):
    """The walrus build in this container encodes at most one sync-wait per
    instruction; Tile's tail drain carries one wait per live semaphore.
    Emit single-wait NOPs on SP instead, then an unguarded drain."""
    gc = tick_clock.global_clock
    ticks = [int(x) for x in re.findall(r"\d+", repr(gc))]
    for i, t in enumerate(ticks):
        if t <= 0:
            continue
        l = [0] * len(ticks)
        l[i] = t
        nop = self.nc.sync.nop(nofuse=True)
        wait_clock.add_sem_waits(nop.ins, ScopedClock({None: bass_rust.VectorClock(l)}))
    self.nc.sync.drain()
    self.nc.all_engine_barrier()
    assert self.sems is not None
    popped = self.nc._tile_sem_poison_stack.pop()
    assert popped is self._sem_poison
    self.nc.clear_and_free_semaphores(list(self.sems.allocated().values()))
    self.nc.all_engine_barrier()


TileContext._drain_and_barrier = _patched_drain_and_barrier


def _split_multi_waits(nc):
    """Move extra sync-waits onto single-wait NOPs inserted just before the
    owning instruction on the same (in-order) engine."""
    for func in nc.m.functions:
        for bb in func.blocks:
            insts = bb.instructions
            if not any(
                i.sync_info is not None
                and i.sync_info.on_wait
                and len(i.sync_info.on_wait) > 1
                for i in insts
            ):
                continue
            new = []
            for inst in insts:
                si = inst.sync_info
                if si is not None and si.on_wait and len(si.on_wait) > 1:
                    waits = list(si.on_wait)
                    for w in waits[:-1]:
                        nop = mybir.InstNoOp(
                            name=nc.get_next_instruction_name(),
                            engine=inst.engine,
                            bass_nofuse=True,
                            sync_info=mybir.SyncInfo(on_wait=[w], on_update=[]),
                        )
                        nc.register_instruction(nop)
                        new.append(nop)
                    inst.sync_info = mybir.SyncInfo(
                        on_wait=[waits[-1]], on_update=list(si.on_update)
                    )
                new.append(inst)
            bb.instructions = new


def _install_ntff_hook():
    """Recreate antenv.axon_hooks (absent in this image) so
    run_bass_kernel_spmd(trace=True) can profile through libaxon_pjrt."""
    if "antenv.axon_hooks" in sys.modules:
        return
    import contextlib
    import ctypes

    mod = types.ModuleType("antenv.axon_hooks")
    _state = {"hook": None}

    def set_axon_ntff_profile_hook(hook):
        _state["hook"] = hook

    def get_axon_ntff_profile_hook():
        return _state["hook"]

    def _ntff_profile_via_ctypes(so_path):
        lib = ctypes.CDLL(so_path)
        if not hasattr(lib, "axon_start_nrt_profile"):
            return None
        lib.axon_start_nrt_profile.argtypes = [
            ctypes.POINTER(ctypes.c_int64),
            ctypes.c_size_t,
        ]
        lib.axon_start_nrt_profile.restype = ctypes.c_int64
        lib.axon_stop_nrt_profile.argtypes = [ctypes.c_char_p]
        lib.axon_stop_nrt_profile.restype = ctypes.c_int64

        @contextlib.contextmanager
        def _hook(output_dir, device_ids):
            import jax

            jax.devices()
            if device_ids:
                ids = (ctypes.c_int64 * len(device_ids))(*device_ids)
                rc = lib.axon_start_nrt_profile(ids, len(device_ids))
            else:
                rc = lib.axon_start_nrt_profile(None, 0)
            if rc != 0:
                raise RuntimeError(f"axon_start_nrt_profile rc={rc}")
            try:
                yield
            finally:
                n = lib.axon_stop_nrt_profile(str(output_dir).encode())
                if n < 0:
                    raise RuntimeError(f"axon_stop_nrt_profile rc={n}")
                print(f"profile: {n} file(s) in {output_dir}", file=sys.stderr)

        return _hook

    mod.set_axon_ntff_profile_hook = set_axon_ntff_profile_hook
    mod.get_axon_ntff_profile_hook = get_axon_ntff_profile_hook
    try:
        set_axon_ntff_profile_hook(
            _ntff_profile_via_ctypes("/opt/axon/libaxon_pjrt.so")
        )
    except Exception:
        pass
    sys.modules["antenv.axon_hooks"] = mod
    try:
        import antenv

        antenv.axon_hooks = mod
    except ImportError:
        pass


_install_ntff_hook()

# ---------------------------------------------------------------------------
# Problem constants
# ---------------------------------------------------------------------------

B, T, C = 4, 2048, 1024
H, HD = 16, 64
NCORES = 8
TQ = T // 2          # q tokens per sub-sweep
NPL = 4              # local head pairs per core (8 heads)
NVG = 2              # V groups of 2 local pairs (4 heads, 256 v dims)
KT_TILES = T // 128  # 16
SCALE = 1.0 / np.sqrt(HD)

F32 = mybir.dt.float32
BF16 = mybir.dt.bfloat16

CC = C // 128  # 8 contraction chunks


# ---------------------------------------------------------------------------
# Device program
# ---------------------------------------------------------------------------

def _build_nc():
    nc = bass.Bass(trn_type="TRN2", target_bir_lowering=False, debug=False)

    xt = nc.declare_dram_parameter("xt", [C, T], BF16, isOutput=False)
    wqt = nc.declare_dram_parameter("wqt", [NPL, 128, CC, 128], BF16,
                                    isOutput=False)
    wkt = nc.declare_dram_parameter("wkt", [NPL, 128, CC, 128], BF16,
                                    isOutput=False)
    wvg = nc.declare_dram_parameter("wvg", [NVG, 128, CC, 256], BF16,
                                    isOutput=False)
    wpt = nc.declare_dram_parameter("wpt", [NPL * 128, C], BF16,
                                    isOutput=False)
    cosk = nc.declare_dram_parameter("cosk", [128, T], BF16, isOutput=False)
    sink = nc.declare_dram_parameter("sink", [128, T], BF16, isOutput=False)
    out_ext = nc.declare_dram_parameter("out", [T, C], BF16,
                                       isOutput=True)

    rs_dram = nc.dram_tensor("rs_scratch", [NPL, 2, 2, TQ], BF16)
    rc_dram = nc.dram_tensor("rc_scratch", [NPL, 2, 2, TQ], BF16)

    with TileContext(nc) as tc:
        with tc.tile_pool(name="persist", bufs=1) as persist:
            qt_sb = persist.tile([128, NPL, T], BF16, tag="qt")
            att_sb = persist.tile([128, NPL, T], BF16, tag="att")
            kt_sb = persist.tile([128, NPL, T], BF16, tag="kt")
            v_sb = persist.tile([128, KT_TILES, 8, 65], BF16, tag="v")
            ck = persist.tile([128, T], BF16, tag="ck")
            sk = persist.tile([128, T], BF16, tag="sk")

            with tc.tile_pool(name="xtpool", bufs=1) as xtpool, \
                 tc.tile_pool(name="pw", bufs=1) as pw:
                xt_sb = xtpool.tile([128, CC, T], BF16, tag="xt")
                xt_r = xt.rearrange("(cc p) t -> p cc t", p=128)
                nc.vector.memset(v_sb[:, :, :, 64:65], 1.0)

                wp_sb = pw.tile([128, NPL, C], BF16)

                _attention(nc, tc, xt_sb, qt_sb, kt_sb, v_sb, att_sb,
                           ck, sk, wqt, wkt, wvg, rs_dram, rc_dram,
                           wp_sb, wpt, xt_r, cosk, sink)

                _phase_proj(nc, tc, wp_sb, att_sb, out_ext)

    _split_multi_waits(nc)
    return nc


def _attention(nc, tc, xt_sb, qt_sb, kt_sb, v_sb, att_sb, ck, sk,
               wqt, wkt, wvg, rs_dram, rc_dram, wp_sb, wpt, xt_r, cosk,
               sink):
    with tc.tile_pool(name="ring", bufs=2, space="PSUM") as ring, \
         tc.tile_pool(name="otps", bufs=1, space="PSUM") as otps, \
         tc.tile_pool(name="wts", bufs=2) as wts, \
         tc.tile_pool(name="qkx", bufs=2) as qkx, \
         tc.tile_pool(name="ptp", bufs=6) as ptp, \
         tc.tile_pool(name="eps", bufs=1) as eps:

        wq_tiles = {}
        wk_tiles = {}
        wv_tiles = {}

        def load_wq(p, eng=None):
            t = wts.tile([128, CC, 128], BF16, tag="wq")
            (eng or nc.gpsimd).dma_start(out=t, in_=wqt[p])
            wq_tiles[p] = t

        def load_wk(p, eng=None):
            t = wts.tile([128, CC, 128], BF16, tag="wk")
            (eng or nc.gpsimd).dma_start(out=t, in_=wkt[p])
            wk_tiles[p] = t

        def load_wv(g, eng=None):
            t = wts.tile([128, CC, 256], BF16, tag="wv")
            if eng == "split":
                nc.scalar.dma_start(out=t[:, 0:4, :], in_=wvg[g, :, 0:4, :])
                nc.gpsimd.dma_start(out=t[:, 4:8, :], in_=wvg[g, :, 4:8, :])
            else:
                (eng or nc.gpsimd).dma_start(out=t, in_=wvg[g])
            wv_tiles[g] = t

        def qk_grant(dst_sb, pair, wtile, col0, swap_eng=None):
            """One 512-token-column projection + rope grant -> dst_sb."""
            if swap_eng is None:
                swap_eng = nc.sync
            ps = ring.tile([128, TQ], F32, tag="ring", name=f"ps_{pair}_{col0}")
            for cc in range(CC):
                nc.tensor.matmul(
                    ps[:, 0:512],
                    lhsT=wtile[:, cc, :],
                    rhs=xt_sb[:, cc, col0:col0 + 512],
                    start=(cc == 0),
                    stop=(cc == CC - 1),
                )
            xb = qkx.tile([128, 512], BF16, tag="xb", name=f"xb_{pair}_{col0}")
            nc.vector.tensor_copy(xb, ps[:, 0:512])
            u = qkx.tile([128, 512], BF16, tag="u", name=f"u_{pair}_{col0}")
            v = qkx.tile([128, 512], BF16, tag="v", name=f"v_{pair}_{col0}")
            vs = qkx.tile([128, 512], BF16, tag="vs", name=f"vs_{pair}_{col0}")
            nc.vector.tensor_mul(u, xb, ck[:, col0:col0 + 512])
            nc.vector.tensor_mul(v, xb, sk[:, col0:col0 + 512])
            for blk in range(4):
                r = blk * 32
                s = (blk ^ 1) * 32
                swap_eng.dma_start(out=vs[r:r + 32, :], in_=v[s:s + 32, :])
            nc.gpsimd.tensor_add(dst_sb[:, pair, col0:col0 + 512], u, vs)

        def v_grant(g, tt):
            """V columns for group g (4 heads), token tile tt."""
            psv = ring.tile([128, 256], F32, tag="ring", name=f"psv_{g}_{tt}")
            for cc in range(CC):
                nc.tensor.matmul(
                    psv,
                    lhsT=xt_sb[:, cc, tt * 128:(tt + 1) * 128],
                    rhs=wv_tiles[g][:, cc, :],
                    start=(cc == 0),
                    stop=(cc == CC - 1),
                )
            nc.vector.tensor_copy(
                v_sb[:, tt, 4 * g:4 * g + 4, 0:64],
                psv.rearrange("p (h d) -> p h d", h=4),
            )

        # ------------------------------------------------------- startup
        load_wq(0, eng=nc.scalar)
        nc.sync.dma_start(out=xt_sb[:, :, 0:512], in_=xt_r[:, :, 0:512])
        load_wk(0, eng=nc.scalar)
        nc.scalar.dma_start(out=ck[:, 0:512], in_=cosk[:, 0:512])
        nc.scalar.dma_start(out=sk[:, 0:512], in_=sink[:, 0:512])
        nc.sync.dma_start(out=xt_sb[:, :, 512:1024], in_=xt_r[:, :, 512:1024])
        load_wv(0, eng="split")
        load_wq(1)
        load_wk(1)
        nc.scalar.dma_start(out=ck[:, 512:1024], in_=cosk[:, 512:1024])
        nc.scalar.dma_start(out=sk[:, 512:1024], in_=sink[:, 512:1024])
        qk_grant(qt_sb, 0, wq_tiles[0], 0, swap_eng=nc.scalar)
        qk_grant(kt_sb, 0, wk_tiles[0], 0, swap_eng=nc.scalar)
        qk_grant(qt_sb, 0, wq_tiles[0], 512, swap_eng=nc.scalar)
        for tt0 in range(6):
            v_grant(0, tt0)
        nc.sync.dma_start(out=xt_sb[:, :, 1024:1536], in_=xt_r[:, :, 1024:1536])
        nc.sync.dma_start(out=xt_sb[:, :, 1536:2048], in_=xt_r[:, :, 1536:2048])
        nc.scalar.dma_start(out=ck[:, TQ:T], in_=cosk[:, TQ:T])
        nc.scalar.dma_start(out=sk[:, TQ:T], in_=sink[:, TQ:T])

        # -------------------------------------------- attention sub-sweeps
        # sub u = (pair p = u//2, q-half s = u%2); q cols qs..qs+1024
        for u in range(2 * NPL):
            p, s = divmod(u, 2)
            qs = s * TQ

            def mk_v(g, tt):
                return lambda: v_grant(g, tt)

            def mk_k(q, n):
                return lambda: qk_grant(kt_sb, q, wk_tiles[q], n * 512)

            def mk_q(q, n):
                return lambda: qk_grant(qt_sb, q, wq_tiles[q], n * 512)

            # Grants allocate PSUM from the same ring as the score
            # tiles; an ODD number of grant allocations in a kt slot
            # shifts the ring phase so the next S tile waits on a
            # same-kt exp (~1-2us stall).  Emit grants in PAIRS.
            fill_at = {}
            if u == 0:
                fill_at = {
                    0: [mk_v(0, 6), mk_v(0, 7)],
                    1: [mk_k(0, 1), mk_k(0, 2)],
                    3: [mk_v(0, 8), mk_v(0, 9)],
                    5: [mk_v(0, 10), mk_v(0, 11)],
                    7: [mk_k(0, 3), mk_v(0, 12)],
                    9: [mk_v(0, 13), mk_v(0, 14)],
                    11: [mk_v(0, 15), mk_q(0, 2)],
                    13: [mk_q(0, 3)],
                }
            elif u in (1, 3, 5):
                q = p + 1
                fill_at = {
                    0: [mk_q(q, 0), mk_q(q, 1)],
                    2: [mk_k(q, 0), mk_k(q, 1)],
                    4: [mk_k(q, 2), mk_k(q, 3)],
                }
                if u == 1:
                    fill_at[6] = [mk_v(1, 0), mk_v(1, 1)]
                    fill_at[8] = [mk_v(1, 2), mk_v(1, 3)]
                    fill_at[10] = [mk_v(1, 4)]
                elif u == 3:
                    fill_at[6] = [mk_v(1, 11), mk_v(1, 12)]
                    fill_at[8] = [mk_v(1, 13), mk_v(1, 14)]
                    fill_at[10] = [mk_v(1, 15)]
            elif u in (2, 4, 6):
                q = p
                fill_at = {0: [mk_q(q, 2), mk_q(q, 3)]}
                if u == 2:
                    fill_at[2] = [mk_v(1, 5), mk_v(1, 6)]
                    fill_at[4] = [mk_v(1, 7), mk_v(1, 8)]
                    fill_at[6] = [mk_v(1, 9), mk_v(1, 10)]

            if u == 1:
                load_wv(1)
            if u == 0:
                load_wq(2)
                load_wk(2)
            if u == 2:
                load_wq(3)
                load_wk(3)
            if u == 6:
                wp_r = wpt.rearrange("(cc p) e -> p cc e", p=128)
                nc.sync.dma_start(out=wp_sb[:, 0:2, :], in_=wp_r[:, 0:2, :])
                nc.sync.dma_start(out=wp_sb[:, 2:4, :], in_=wp_r[:, 2:4, :])

            psAB = otps.tile([128, 2, TQ], F32, tag="ot", name=f"psAB_{u}")

            def ot_mm_A(kt, pt0, pt1):
                nc.tensor.matmul(
                    psAB[0:65, 0, 0:512],
                    lhsT=v_sb[:, kt, 2 * p, :],
                    rhs=pt0[:, 0:512],
                    start=(kt == 0),
                    stop=(kt == KT_TILES - 1),
                )
                nc.tensor.matmul(
                    psAB[0:65, 0, 512:1024],
                    lhsT=v_sb[:, kt, 2 * p, :],
                    rhs=pt1[:, 0:512],
                    start=(kt == 0),
                    stop=(kt == KT_TILES - 1),
                )

            def ot_mm_B(kt, pt0, pt1):
                nc.tensor.matmul(
                    psAB[0:65, 1, 0:512],
                    lhsT=v_sb[:, kt, 2 * p + 1, :],
                    rhs=pt0[:, 512:1024],
                    start=(kt == 0),
                    stop=(kt == KT_TILES - 1),
                )
                nc.tensor.matmul(
                    psAB[0:65, 1, 512:1024],
                    lhsT=v_sb[:, kt, 2 * p + 1, :],
                    rhs=pt1[:, 512:1024],
                    start=(kt == 0),
                    stop=(kt == KT_TILES - 1),
                )

            pends = []
            for kt in range(KT_TILES):
                st0 = ring.tile([128, TQ], F32, tag="ring",
                                name=f"st0_{u}_{kt}")
                st1 = ring.tile([128, TQ], F32, tag="ring",
                                name=f"st1_{u}_{kt}")
                nc.tensor.matmul(
                    st0[:, 0:512],
                    lhsT=kt_sb[0:64, p, kt * 128:(kt + 1) * 128],
                    rhs=qt_sb[0:64, p, qs:qs + 512],
                    start=True, stop=True, tile_position=(0, 0),
                )
                nc.tensor.matmul(
                    st0[:, 512:1024],
                    lhsT=kt_sb[64:128, p, kt * 128:(kt + 1) * 128],
                    rhs=qt_sb[64:128, p, qs:qs + 512],
                    start=True, stop=True, tile_position=(64, 0),
                )
                pt0 = ptp.tile([128, TQ], BF16, tag="pt",
                               name=f"pt0_{u}_{kt}")
                nc.scalar.activation(
                    out=pt0, in_=st0,
                    func=mybir.ActivationFunctionType.Exp, scale=SCALE,
                )
                if len(pends) == 2:
                    ot_mm_A(*pends[0])
                nc.tensor.matmul(
                    st1[:, 0:512],
                    lhsT=kt_sb[0:64, p, kt * 128:(kt + 1) * 128],
                    rhs=qt_sb[0:64, p, qs + 512:qs + 1024],
                    start=True, stop=True, tile_position=(0, 0),
                )
                nc.tensor.matmul(
                    st1[:, 512:1024],
                    lhsT=kt_sb[64:128, p, kt * 128:(kt + 1) * 128],
                    rhs=qt_sb[64:128, p, qs + 512:qs + 1024],
                    start=True, stop=True, tile_position=(64, 0),
                )
                pt1 = ptp.tile([128, TQ], BF16, tag="pt",
                               name=f"pt1_{u}_{kt}")
                nc.scalar.activation(
                    out=pt1, in_=st1,
                    func=mybir.ActivationFunctionType.Exp, scale=SCALE,
                )
                if len(pends) == 2:
                    ot_mm_B(*pends.pop(0))
                pends.append((kt, pt0, pt1))
                for f in fill_at.get(kt, ()):
                    f()
            for e in pends:
                ot_mm_A(*e)
                ot_mm_B(*e)

            # --------------------------------------------- sub epilogue
            attU = eps.tile([65, 2, TQ], BF16, tag="attU", bufs=1,
                            name=f"attU_{u}")
            nc.vector.tensor_copy(attU, psAB[0:65, :, :])
            nc.scalar.dma_start(out=rs_dram[p, s], in_=attU[64:65, :, :])
            den128 = eps.tile([128, 16], BF16, tag="den128", bufs=1,
                              name=f"den128_{u}")
            nc.sync.dma_start(out=den128, in_=rs_dram[p, s])
            denf = eps.tile([128, 16], F32, tag="denf", bufs=1,
                            name=f"denf_{u}")
            nc.vector.tensor_copy(denf, den128)
            recf = eps.tile([128, 16], F32, tag="recf", bufs=1,
                            name=f"recf_{u}")
            nc.vector.reciprocal(recf, denf)
            rec128 = eps.tile([128, 16], BF16, tag="rec128", bufs=1,
                              name=f"rec128_{u}")
            nc.vector.tensor_copy(rec128, recf)
            nc.scalar.dma_start(out=rc_dram[p, s], in_=rec128)
            rbc = eps.tile([64, 2, TQ], BF16, tag="rbc", bufs=1,
                           name=f"rbc_{u}")
            nc.sync.dma_start(
                out=rbc[:, 0, :],
                in_=rc_dram[p, s, 0:1, :].broadcast_to([64, TQ]))
            nc.scalar.dma_start(
                out=rbc[:, 1, :],
                in_=rc_dram[p, s, 1:2, :].broadcast_to([64, TQ]))
            attB = eps.tile([64, TQ], BF16, tag="attB", bufs=1,
                            name=f"attB_{u}")
            nc.vector.tensor_mul(att_sb[0:64, p, qs:qs + TQ],
                                 attU[0:64, 0, :], rbc[:, 0, :])
            nc.vector.tensor_mul(attB, attU[0:64, 1, :], rbc[:, 1, :])
            nc.sync.dma_start(out=att_sb[64:128, p, qs:qs + TQ], in_=attB)


def _phase_proj(nc, tc, wp_sb, att_sb, out_ext):
    """out_partial = att^T @ WpT-rows (my 512 channels), per 128-token
    tile over the full 2048 tokens."""
    with tc.tile_pool(name="pph", bufs=2) as pph, \
         tc.tile_pool(name="pps", bufs=4, space="PSUM") as pps:
        NTT = T // 128

        def head_mm(ps, tt):
            for lp in range(NPL - 1):
                for nch in range(2):
                    nc.tensor.matmul(
                        ps[:, nch * 512:(nch + 1) * 512],
                        lhsT=att_sb[:, lp, tt * 128:(tt + 1) * 128],
                        rhs=wp_sb[:, lp, nch * 512:(nch + 1) * 512],
                        start=(lp == 0),
                        stop=False,
                    )

        def tail_mm(ps, tt):
            for nch in range(2):
                nc.tensor.matmul(
                    ps[:, nch * 512:(nch + 1) * 512],
                    lhsT=att_sb[:, NPL - 1, tt * 128:(tt + 1) * 128],
                    rhs=wp_sb[:, NPL - 1, nch * 512:(nch + 1) * 512],
                    start=False,
                    stop=True,
                )
            o = pph.tile([128, C], BF16, tag="o", name=f"o_{tt}")
            if tt % 2 == 0:
                nc.vector.tensor_copy(o, ps)
            else:
                nc.scalar.activation(
                    out=o, in_=ps, func=mybir.ActivationFunctionType.Copy
                )
            r = tt * 128
            nc.sync.dma_start(out=out_ext[r:r + 128, 0:512], in_=o[:, 0:512])
            nc.scalar.dma_start(out=out_ext[r:r + 128, 512:1024],
                                in_=o[:, 512:1024])

        pend = []
        for tt in range(NTT):
            ps = pps.tile([128, C], F32, tag="ps", name=f"ps_{tt}")
            head_mm(ps, tt)
            pend.append((ps, tt))
            if len(pend) > 2:
                tail_mm(*pend.pop(0))
        for e in pend:
            tail_mm(*e)


_NC_CACHE = None


def _get_nc():
    global _NC_CACHE
    if _NC_CACHE is None:
        _NC_CACHE = _build_nc()
    return _NC_CACHE


# ---------------------------------------------------------------------------
# Host wrapper
# ---------------------------------------------------------------------------

def kernel(x, W_qkv, W_proj, cos, sin, mask):
    bf = ml_dtypes.bfloat16
    x = np.asarray(x, dtype=np.float32)
    W_qkv = np.asarray(W_qkv, dtype=np.float32)
    W_proj = np.asarray(W_proj, dtype=np.float32)
    cos = np.asarray(cos, dtype=np.float32)
    sin = np.asarray(sin, dtype=np.float32)

    # Permute q/k head dims: interleaved (x1,x2 pairs) -> halves [x1; x2].
    perm = np.concatenate([np.arange(0, HD, 2), np.arange(1, HD, 2)])
    Wq = W_qkv[0:C].reshape(H, HD, C)[:, perm, :].reshape(C, C)
    Wk = W_qkv[C:2 * C].reshape(H, HD, C)[:, perm, :].reshape(C, C)
    Wv = W_qkv[2 * C:3 * C]

    # full-H tiled layouts: [8 pairs, 128 c-part, CC, 128 d]
    wqt_full = np.ascontiguousarray(
        Wq.T.astype(bf).reshape(CC, 128, 8, 128).transpose(2, 1, 0, 3)
    )
    wkt_full = np.ascontiguousarray(
        Wk.T.astype(bf).reshape(CC, 128, 8, 128).transpose(2, 1, 0, 3)
    )
    # V weights in 4-head group slabs: [4 groups, 128 c-part, CC, 256]
    wvg_full = np.ascontiguousarray(
        Wv.T.astype(bf).reshape(CC, 128, 4, 256).transpose(2, 1, 0, 3)
    )
    wpt_full = W_proj.T.astype(bf)   # [1024 rows (channels), 1024]

    cosT = cos.T
    sinT = sin.T
    cosr = np.ascontiguousarray(np.tile(cosT, (4, 1)).astype(bf))
    sinB = np.ascontiguousarray(
        np.tile(np.concatenate([sinT, -sinT], axis=0), (2, 1)).astype(bf)
    )

    in_maps = []
    for c in range(NCORES):
        b, hh = divmod(c, 2)
        xtb = np.ascontiguousarray(x[b].T.astype(bf))
        in_maps.append(
            {
                "xt": xtb,
                "wqt": np.ascontiguousarray(wqt_full[4 * hh:4 * hh + 4]),
                "wkt": np.ascontiguousarray(wkt_full[4 * hh:4 * hh + 4]),
                "wvg": np.ascontiguousarray(wvg_full[2 * hh:2 * hh + 2]),
                "wpt": np.ascontiguousarray(
                    wpt_full[512 * hh:512 * hh + 512]),
                "cosk": cosr,
                "sink": sinB,
            }
        )

    nc = _get_nc()
    trace = bool(int(os.environ.get("BASSK_TRACE", "0")))
    res = bass_utils.run_bass_kernel_spmd(
        nc, in_maps, core_ids=list(range(NCORES)), trace=trace
    )
    if trace:
        kernel.last_exec_time_ns = res.exec_time_ns
        kernel.last_profile = res

    out = np.empty((B, T, C), dtype=np.float32)
    for b in range(B):
        out[b] = (res.results[2 * b]["out"].astype(np.float32)
                  + res.results[2 * b + 1]["out"].astype(np.float32))
    return out


# revision 19
# speedup vs baseline: 1.0281x; 1.0281x over previous
"""Distributed Trainium2 Bass kernel for nn_Attention_62766652063769 (v6).

Reference computation (B=4, T=2048, C=1024, H=16, HD=64):
    qkv = x @ W_qkv^T ; split into q, k, v heads
    q, k <- RoPE(q), RoPE(k)   (interleaved-pair rotation)
    attn = softmax(q k^T / sqrt(HD))   (mask is all-ones -> no masking)
    out  = (attn @ v) @ W_proj^T

Sharding (tensor-parallel over heads, per the sharding hint): core
c = 2*b + hh owns batch b and head-half hh (8 of 16 heads), over the
FULL 2048 query tokens.  Q/K/V projections and attention are computed
only for the core's own heads (no redundant K/V work); the output
projection is row-sharded over the core's 512 att channels, producing
a bf16 PARTIAL result per core which the host sums per batch at
gather time (out[b] = partial[2b] + partial[2b+1]).

Device structure per core (4 local head pairs x 2 query-half
sub-sweeps of 1024 q tokens, identical inner loop each):
  - Merged score tiles: one PSUM tile [128, 1024] per (kt, q-chunk)
    holds BOTH heads side by side via tile_position (0,0)/(64,0) ->
    row-tiled concurrent matmuls, exp starts after 2 MMs.
  - ACT runs ONLY Exp (one table load).  Softmax denominator
    reciprocal is computed on a compact [128, 16] reshape (DRAM
    round-trip) so the slow DVE reciprocal touches 16 elems/lane.
  - OT (attn @ V) deferred 2 kt-tiles, A,A,B,B stationary reuse,
    65th V column accumulates the softmax denominator.
  - Filler grants (next Q/K/V projections + rope) are interleaved
    into the ACT-bound attention loop in PAIRS (grant PSUM tiles
    share the score ring; odd counts shift its recycle phase).
  - Proj phase: per 128-token tile, head/tail split so the last
    pair's late-normalized att never blocks the in-order PE queue;
    bf16 output halves DMA'd on sync+scalar queues.
"""

import os
import re
import sys
import types

if "/opt/trn_rl_repo" not in sys.path:
    sys.path.insert(0, "/opt/trn_rl_repo")

import ml_dtypes
import numpy as np

import bass_rust
import concourse.bass as bass
import concourse.mybir as mybir
from concourse import bass_utils
from concourse.tile import TileContext, ScopedClock

# ---------------------------------------------------------------------------
# Environment patches (same as v1/v2)
# ---------------------------------------------------------------------------

def _patched_drain_and_barrier(self, tick_clock, wait_clock):
    """The walrus build in this container encodes at most one sync-wait per
    instruction; Tile's tail drain carries one wait per live semaphore.
    Emit single-wait NOPs on SP instead, then an unguarded drain."""
    gc = tick_clock.global_clock
    ticks = [int(x) for x in re.findall(r"\d+", repr(gc))]
    for i, t in enumerate(ticks):
        if t <= 0:
            continue
        l = [0] * len(ticks)
        l[i] = t
        nop = self.nc.sync.nop(nofuse=True)
        wait_clock.add_sem_waits(nop.ins, ScopedClock({None: bass_rust.VectorClock(l)}))
    self.nc.sync.drain()
    self.nc.all_engine_barrier()
    assert self.sems is not None
    popped = self.nc._tile_sem_poison_stack.pop()
    assert popped is self._sem_poison
    self.nc.clear_and_free_semaphores(list(self.sems.allocated().values()))
    self.nc.all_engine_barrier()


TileContext._drain_and_barrier = _patched_drain_and_barrier


def _split_multi_waits(nc):
    """Move extra sync-waits onto single-wait NOPs inserted just before the
    owning instruction on the same (in-order) engine."""
    for func in nc.m.functions:
        for bb in func.blocks:
            insts = bb.instructions
            if not any(
                i.sync_info is not None
                and i.sync_info.on_wait
                and len(i.sync_info.on_wait) > 1
                for i in insts
            ):
                continue
            new = []
            for inst in insts:
                si = inst.sync_info
                if si is not None and si.on_wait and len(si.on_wait) > 1:
                    waits = list(si.on_wait)
                    for w in waits[:-1]:
                        nop = mybir.InstNoOp(
                            name=nc.get_next_instruction_name(),
                            engine=inst.engine,
                            bass_nofuse=True,
                            sync_info=mybir.SyncInfo(on_wait=[w], on_update=[]),
                        )
                        nc.register_instruction(nop)
                        new.append(nop)
                    inst.sync_info = mybir.SyncInfo(
                        on_wait=[waits[-1]], on_update=list(si.on_update)
                    )
                new.append(inst)
            bb.instructions = new


def _install_ntff_hook():
    """Recreate antenv.axon_hooks (absent in this image) so
    run_bass_kernel_spmd(trace=True) can profile through libaxon_pjrt."""
    if "antenv.axon_hooks" in sys.modules:
        return
    import contextlib
    import ctypes

    mod = types.ModuleType("antenv.axon_hooks")
    _state = {"hook": None}

    def set_axon_ntff_profile_hook(hook):
        _state["hook"] = hook

    def get_axon_ntff_profile_hook():
        return _state["hook"]

    def _ntff_profile_via_ctypes(so_path):
        lib = ctypes.CDLL(so_path)
        if not hasattr(lib, "axon_start_nrt_profile"):
            return None
        lib.axon_start_nrt_profile.argtypes = [
            ctypes.POINTER(ctypes.c_int64),
            ctypes.c_size_t,
        ]
        lib.axon_start_nrt_profile.restype = ctypes.c_int64
        lib.axon_stop_nrt_profile.argtypes = [ctypes.c_char_p]
        lib.axon_stop_nrt_profile.restype = ctypes.c_int64

        @contextlib.contextmanager
        def _hook(output_dir, device_ids):
            import jax

            jax.devices()
            if device_ids:
                ids = (ctypes.c_int64 * len(device_ids))(*device_ids)
                rc = lib.axon_start_nrt_profile(ids, len(device_ids))
            else:
                rc = lib.axon_start_nrt_profile(None, 0)
            if rc != 0:
                raise RuntimeError(f"axon_start_nrt_profile rc={rc}")
            try:
                yield
            finally:
                n = lib.axon_stop_nrt_profile(str(output_dir).encode())
                if n < 0:
                    raise RuntimeError(f"axon_stop_nrt_profile rc={n}")
                print(f"profile: {n} file(s) in {output_dir}", file=sys.stderr)

        return _hook

    mod.set_axon_ntff_profile_hook = set_axon_ntff_profile_hook
    mod.get_axon_ntff_profile_hook = get_axon_ntff_profile_hook
    try:
        set_axon_ntff_profile_hook(
            _ntff_profile_via_ctypes("/opt/axon/libaxon_pjrt.so")
        )
    except Exception:
        pass
    sys.modules["antenv.axon_hooks"] = mod
    try:
        import antenv

        antenv.axon_hooks = mod
    except ImportError:
        pass


_install_ntff_hook()

# ---------------------------------------------------------------------------
# Problem constants
# ---------------------------------------------------------------------------

B, T, C = 4, 2048, 1024
H, HD = 16, 64
NCORES = 8
TQ = T // 2          # q tokens per sub-sweep
NPL = 4              # local head pairs per core (8 heads)
NVG = 2              # V groups of 2 local pairs (4 heads, 256 v dims)
KT_TILES = T // 128  # 16
SCALE = 1.0 / np.sqrt(HD)

F32 = mybir.dt.float32
BF16 = mybir.dt.bfloat16

CC = C // 128  # 8 contraction chunks


# ---------------------------------------------------------------------------
# Device program
# ---------------------------------------------------------------------------

def _build_nc():
    nc = bass.Bass(trn_type="TRN2", target_bir_lowering=False, debug=False)

    xt = nc.declare_dram_parameter("xt", [C, T], BF16, isOutput=False)
    wqt = nc.declare_dram_parameter("wqt", [NPL, 128, CC, 128], BF16,
                                    isOutput=False)
    wkt = nc.declare_dram_parameter("wkt", [NPL, 128, CC, 128], BF16,
                                    isOutput=False)
    wvg = nc.declare_dram_parameter("wvg", [NVG, 128, CC, 256], BF16,
                                    isOutput=False)
    wpt = nc.declare_dram_parameter("wpt", [NPL * 128, C], BF16,
                                    isOutput=False)
    cosk = nc.declare_dram_parameter("cosk", [128, T], BF16, isOutput=False)
    sink = nc.declare_dram_parameter("sink", [128, T], BF16, isOutput=False)
    out_ext = nc.declare_dram_parameter("out", [T, C], BF16,
                                       isOutput=True)

    rs_dram = nc.dram_tensor("rs_scratch", [NPL, 2, 2, TQ], BF16)
    rc_dram = nc.dram_tensor("rc_scratch", [NPL, 2, 2, TQ], BF16)

    with TileContext(nc) as tc:
        with tc.tile_pool(name="persist", bufs=1) as persist:
            qt_sb = persist.tile([128, NPL, T], BF16, tag="qt")
            att_sb = persist.tile([128, NPL, T], BF16, tag="att")
            kt_sb = persist.tile([128, NPL, T], BF16, tag="kt")
            v_sb = persist.tile([128, KT_TILES, 8, 65], BF16, tag="v")
            ck = persist.tile([128, T], BF16, tag="ck")
            sk = persist.tile([128, T], BF16, tag="sk")

            with tc.tile_pool(name="xtpool", bufs=1) as xtpool, \
                 tc.tile_pool(name="pw", bufs=1) as pw:
                xt_sb = xtpool.tile([128, CC, T], BF16, tag="xt")
                xt_r = xt.rearrange("(cc p) t -> p cc t", p=128)
                nc.vector.memset(v_sb[:, :, :, 64:65], 1.0)

                wp_sb = pw.tile([128, NPL, C], BF16)

                _attention(nc, tc, xt_sb, qt_sb, kt_sb, v_sb, att_sb,
                           ck, sk, wqt, wkt, wvg, rs_dram, rc_dram,
                           wp_sb, wpt, xt_r, cosk, sink)

                _phase_proj(nc, tc, wp_sb, att_sb, out_ext)

    _split_multi_waits(nc)
    return nc


def _attention(nc, tc, xt_sb, qt_sb, kt_sb, v_sb, att_sb, ck, sk,
               wqt, wkt, wvg, rs_dram, rc_dram, wp_sb, wpt, xt_r, cosk,
               sink):
    with tc.tile_pool(name="ring", bufs=2, space="PSUM") as ring, \
         tc.tile_pool(name="otps", bufs=1, space="PSUM") as otps, \
         tc.tile_pool(name="wts", bufs=2) as wts, \
         tc.tile_pool(name="qkx", bufs=2) as qkx, \
         tc.tile_pool(name="ptp", bufs=6) as ptp, \
         tc.tile_pool(name="eps", bufs=1) as eps:

        wq_tiles = {}
        wk_tiles = {}
        wv_tiles = {}

        def load_wq(p, eng=None):
            t = wts.tile([128, CC, 128], BF16, tag="wq")
            (eng or nc.gpsimd).dma_start(out=t, in_=wqt[p])
            wq_tiles[p] = t

        def load_wk(p, eng=None):
            t = wts.tile([128, CC, 128], BF16, tag="wk")
            (eng or nc.gpsimd).dma_start(out=t, in_=wkt[p])
            wk_tiles[p] = t

        def load_wv(g, eng=None):
            t = wts.tile([128, CC, 256], BF16, tag="wv")
            (eng or nc.gpsimd).dma_start(out=t, in_=wvg[g])
            wv_tiles[g] = t

        def qk_grant(dst_sb, pair, wtile, col0, swap_eng=None):
            """One 512-token-column projection + rope grant -> dst_sb."""
            if swap_eng is None:
                swap_eng = nc.sync
            ps = ring.tile([128, TQ], F32, tag="ring", name=f"ps_{pair}_{col0}")
            for cc in range(CC):
                nc.tensor.matmul(
                    ps[:, 0:512],
                    lhsT=wtile[:, cc, :],
                    rhs=xt_sb[:, cc, col0:col0 + 512],
                    start=(cc == 0),
                    stop=(cc == CC - 1),
                )
            xb = qkx.tile([128, 512], BF16, tag="xb", name=f"xb_{pair}_{col0}")
            nc.vector.tensor_copy(xb, ps[:, 0:512])
            u = qkx.tile([128, 512], BF16, tag="u", name=f"u_{pair}_{col0}")
            v = qkx.tile([128, 512], BF16, tag="v", name=f"v_{pair}_{col0}")
            vs = qkx.tile([128, 512], BF16, tag="vs", name=f"vs_{pair}_{col0}")
            nc.vector.tensor_mul(u, xb, ck[:, col0:col0 + 512])
            nc.vector.tensor_mul(v, xb, sk[:, col0:col0 + 512])
            for blk in range(4):
                r = blk * 32
                s = (blk ^ 1) * 32
                swap_eng.dma_start(out=vs[r:r + 32, :], in_=v[s:s + 32, :])
            nc.gpsimd.tensor_add(dst_sb[:, pair, col0:col0 + 512], u, vs)

        def v_grant(g, tt):
            """V columns for group g (4 heads), token tile tt."""
            psv = ring.tile([128, 256], F32, tag="ring", name=f"psv_{g}_{tt}")
            for cc in range(CC):
                nc.tensor.matmul(
                    psv,
                    lhsT=xt_sb[:, cc, tt * 128:(tt + 1) * 128],
                    rhs=wv_tiles[g][:, cc, :],
                    start=(cc == 0),
                    stop=(cc == CC - 1),
                )
            nc.vector.tensor_copy(
                v_sb[:, tt, 4 * g:4 * g + 4, 0:64],
                psv.rearrange("p (h d) -> p h d", h=4),
            )

        # ------------------------------------------------------- startup
        load_wq(0, eng=nc.scalar)
        nc.sync.dma_start(out=xt_sb[:, :, 0:512], in_=xt_r[:, :, 0:512])
        load_wk(0, eng=nc.scalar)
        nc.scalar.dma_start(out=ck[:, 0:512], in_=cosk[:, 0:512])
        nc.scalar.dma_start(out=sk[:, 0:512], in_=sink[:, 0:512])
        nc.sync.dma_start(out=xt_sb[:, :, 512:1024], in_=xt_r[:, :, 512:1024])
        load_wv(0, eng=nc.gpsimd)
        load_wq(1)
        load_wk(1)
        nc.scalar.dma_start(out=ck[:, 512:1024], in_=cosk[:, 512:1024])
        nc.scalar.dma_start(out=sk[:, 512:1024], in_=sink[:, 512:1024])
        nc.sync.dma_start(out=xt_sb[:, :, 1024:1536], in_=xt_r[:, :, 1024:1536])
        qk_grant(qt_sb, 0, wq_tiles[0], 0, swap_eng=nc.scalar)
        qk_grant(kt_sb, 0, wk_tiles[0], 0, swap_eng=nc.scalar)
        qk_grant(qt_sb, 0, wq_tiles[0], 512, swap_eng=nc.scalar)
        for tt0 in range(6):
            v_grant(0, tt0)
        nc.sync.dma_start(out=xt_sb[:, :, 1536:2048], in_=xt_r[:, :, 1536:2048])
        nc.scalar.dma_start(out=ck[:, TQ:T], in_=cosk[:, TQ:T])
        nc.scalar.dma_start(out=sk[:, TQ:T], in_=sink[:, TQ:T])

        # -------------------------------------------- attention sub-sweeps
        # sub u = (pair p = u//2, q-half s = u%2); q cols qs..qs+1024
        for u in range(2 * NPL):
            p, s = divmod(u, 2)
            qs = s * TQ

            def mk_v(g, tt):
                return lambda: v_grant(g, tt)

            def mk_k(q, n):
                return lambda: qk_grant(kt_sb, q, wk_tiles[q], n * 512)

            def mk_q(q, n):
                return lambda: qk_grant(qt_sb, q, wq_tiles[q], n * 512)

            # Grants allocate PSUM from the same ring as the score
            # tiles; an ODD number of grant allocations in a kt slot
            # shifts the ring phase so the next S tile waits on a
            # same-kt exp (~1-2us stall).  Emit grants in PAIRS.
            fill_at = {}
            if u == 0:
                fill_at = {
                    0: [mk_v(0, 6), mk_v(0, 7)],
                    1: [mk_k(0, 1), mk_k(0, 2)],
                    3: [mk_v(0, 8), mk_v(0, 9)],
                    5: [mk_v(0, 10), mk_v(0, 11)],
                    7: [mk_v(0, 12), mk_v(0, 13)],
                    9: [mk_k(0, 3), mk_v(0, 14)],
                    11: [mk_v(0, 15), mk_q(0, 2)],
                    13: [mk_q(0, 3)],
                }
            elif u in (1, 3, 5):
                q = p + 1
                fill_at = {
                    0: [mk_q(q, 0), mk_q(q, 1)],
                    2: [mk_k(q, 0), mk_k(q, 1)],
                    4: [mk_k(q, 2), mk_k(q, 3)],
                }
                if u == 1:
                    fill_at[6] = [mk_v(1, 0), mk_v(1, 1)]
                    fill_at[8] = [mk_v(1, 2), mk_v(1, 3)]
                    fill_at[10] = [mk_v(1, 4)]
                elif u == 3:
                    fill_at[6] = [mk_v(1, 11), mk_v(1, 12)]
                    fill_at[8] = [mk_v(1, 13), mk_v(1, 14)]
                    fill_at[10] = [mk_v(1, 15)]
            elif u in (2, 4, 6):
                q = p
                fill_at = {0: [mk_q(q, 2), mk_q(q, 3)]}
                if u == 2:
                    fill_at[2] = [mk_v(1, 5), mk_v(1, 6)]
                    fill_at[4] = [mk_v(1, 7), mk_v(1, 8)]
                    fill_at[6] = [mk_v(1, 9), mk_v(1, 10)]

            if u == 1:
                load_wv(1)
            if u == 0:
                load_wq(2)
                load_wk(2)
            if u == 2:
                load_wq(3)
                load_wk(3)
            if u == 6:
                wp_r = wpt.rearrange("(cc p) e -> p cc e", p=128)
                nc.sync.dma_start(out=wp_sb[:, 0:2, :], in_=wp_r[:, 0:2, :])
                nc.sync.dma_start(out=wp_sb[:, 2:4, :], in_=wp_r[:, 2:4, :])

            psAB = otps.tile([128, 2, TQ], F32, tag="ot", name=f"psAB_{u}")

            def ot_mm_A(kt, pt0, pt1):
                nc.tensor.matmul(
                    psAB[0:65, 0, 0:512],
                    lhsT=v_sb[:, kt, 2 * p, :],
                    rhs=pt0[:, 0:512],
                    start=(kt == 0),
                    stop=(kt == KT_TILES - 1),
                )
                nc.tensor.matmul(
                    psAB[0:65, 0, 512:1024],
                    lhsT=v_sb[:, kt, 2 * p, :],
                    rhs=pt1[:, 0:512],
                    start=(kt == 0),
                    stop=(kt == KT_TILES - 1),
                )

            def ot_mm_B(kt, pt0, pt1):
                nc.tensor.matmul(
                    psAB[0:65, 1, 0:512],
                    lhsT=v_sb[:, kt, 2 * p + 1, :],
                    rhs=pt0[:, 512:1024],
                    start=(kt == 0),
                    stop=(kt == KT_TILES - 1),
                )
                nc.tensor.matmul(
                    psAB[0:65, 1, 512:1024],
                    lhsT=v_sb[:, kt, 2 * p + 1, :],
                    rhs=pt1[:, 512:1024],
                    start=(kt == 0),
                    stop=(kt == KT_TILES - 1),
                )

            pends = []
            for kt in range(KT_TILES):
                st0 = ring.tile([128, TQ], F32, tag="ring",
                                name=f"st0_{u}_{kt}")
                st1 = ring.tile([128, TQ], F32, tag="ring",
                                name=f"st1_{u}_{kt}")
                nc.tensor.matmul(
                    st0[:, 0:512],
                    lhsT=kt_sb[0:64, p, kt * 128:(kt + 1) * 128],
                    rhs=qt_sb[0:64, p, qs:qs + 512],
                    start=True, stop=True, tile_position=(0, 0),
                )
                nc.tensor.matmul(
                    st0[:, 512:1024],
                    lhsT=kt_sb[64:128, p, kt * 128:(kt + 1) * 128],
                    rhs=qt_sb[64:128, p, qs:qs + 512],
                    start=True, stop=True, tile_position=(64, 0),
                )
                pt0 = ptp.tile([128, TQ], BF16, tag="pt",
                               name=f"pt0_{u}_{kt}")
                nc.scalar.activation(
                    out=pt0, in_=st0,
                    func=mybir.ActivationFunctionType.Exp, scale=SCALE,
                )
                if len(pends) == 2:
                    ot_mm_A(*pends[0])
                nc.tensor.matmul(
                    st1[:, 0:512],
                    lhsT=kt_sb[0:64, p, kt * 128:(kt + 1) * 128],
                    rhs=qt_sb[0:64, p, qs + 512:qs + 1024],
                    start=True, stop=True, tile_position=(0, 0),
                )
                nc.tensor.matmul(
                    st1[:, 512:1024],
                    lhsT=kt_sb[64:128, p, kt * 128:(kt + 1) * 128],
                    rhs=qt_sb[64:128, p, qs + 512:qs + 1024],
                    start=True, stop=True, tile_position=(64, 0),
                )
                pt1 = ptp.tile([128, TQ], BF16, tag="pt",
                               name=f"pt1_{u}_{kt}")
                nc.scalar.activation(
                    out=pt1, in_=st1,
                    func=mybir.ActivationFunctionType.Exp, scale=SCALE,
                )
                if len(pends) == 2:
                    ot_mm_B(*pends.pop(0))
                pends.append((kt, pt0, pt1))
                for f in fill_at.get(kt, ()):
                    f()
            for e in pends:
                ot_mm_A(*e)
                ot_mm_B(*e)

            # --------------------------------------------- sub epilogue
            attU = eps.tile([65, 2, TQ], BF16, tag="attU", bufs=1,
                            name=f"attU_{u}")
            nc.vector.tensor_copy(attU, psAB[0:65, :, :])
            nc.sync.dma_start(out=rs_dram[p, s], in_=attU[64:65, :, :])
            den128 = eps.tile([128, 16], BF16, tag="den128", bufs=1,
                              name=f"den128_{u}")
            nc.sync.dma_start(out=den128, in_=rs_dram[p, s])
            denf = eps.tile([128, 16], F32, tag="denf", bufs=1,
                            name=f"denf_{u}")
            nc.vector.tensor_copy(denf, den128)
            recf = eps.tile([128, 16], F32, tag="recf", bufs=1,
                            name=f"recf_{u}")
            nc.vector.reciprocal(recf, denf)
            rec128 = eps.tile([128, 16], BF16, tag="rec128", bufs=1,
                              name=f"rec128_{u}")
            nc.vector.tensor_copy(rec128, recf)
            nc.sync.dma_start(out=rc_dram[p, s], in_=rec128)
            rbc = eps.tile([64, 2, TQ], BF16, tag="rbc", bufs=1,
                           name=f"rbc_{u}")
            nc.sync.dma_start(
                out=rbc[:, 0, :],
                in_=rc_dram[p, s, 0:1, :].broadcast_to([64, TQ]))
            nc.sync.dma_start(
                out=rbc[:, 1, :],
                in_=rc_dram[p, s, 1:2, :].broadcast_to([64, TQ]))
            attB = eps.tile([64, TQ], BF16, tag="attB", bufs=1,
                            name=f"attB_{u}")
            nc.vector.tensor_mul(att_sb[0:64, p, qs:qs + TQ],
                                 attU[0:64, 0, :], rbc[:, 0, :])
            nc.vector.tensor_mul(attB, attU[0:64, 1, :], rbc[:, 1, :])
            nc.sync.dma_start(out=att_sb[64:128, p, qs:qs + TQ], in_=attB)


def _phase_proj(nc, tc, wp_sb, att_sb, out_ext):
    """out_partial = att^T @ WpT-rows (my 512 channels), per 128-token
    tile over the full 2048 tokens."""
    with tc.tile_pool(name="pph", bufs=2) as pph, \
         tc.tile_pool(name="pps", bufs=4, space="PSUM") as pps:
        NTT = T // 128

        def head_mm(ps, tt):
            for lp in range(NPL - 1):
                for nch in range(2):
                    nc.tensor.matmul(
                        ps[:, nch * 512:(nch + 1) * 512],
                        lhsT=att_sb[:, lp, tt * 128:(tt + 1) * 128],
                        rhs=wp_sb[:, lp, nch * 512:(nch + 1) * 512],
                        start=(lp == 0),
                        stop=False,
                    )

        def tail_mm(ps, tt):
            for nch in range(2):
                nc.tensor.matmul(
                    ps[:, nch * 512:(nch + 1) * 512],
                    lhsT=att_sb[:, NPL - 1, tt * 128:(tt + 1) * 128],
                    rhs=wp_sb[:, NPL - 1, nch * 512:(nch + 1) * 512],
                    start=False,
                    stop=True,
                )
            o = pph.tile([128, C], BF16, tag="o", name=f"o_{tt}")
            if tt % 2 == 0:
                nc.vector.tensor_copy(o, ps)
            else:
                nc.scalar.activation(
                    out=o, in_=ps, func=mybir.ActivationFunctionType.Copy
                )
            r = tt * 128
            nc.sync.dma_start(out=out_ext[r:r + 128, 0:512], in_=o[:, 0:512])
            nc.scalar.dma_start(out=out_ext[r:r + 128, 512:1024],
                                in_=o[:, 512:1024])

        pend = []
        for tt in range(NTT):
            ps = pps.tile([128, C], F32, tag="ps", name=f"ps_{tt}")
            head_mm(ps, tt)
            pend.append((ps, tt))
            if len(pend) > 2:
                tail_mm(*pend.pop(0))
        for e in pend:
            tail_mm(*e)


_NC_CACHE = None


def _get_nc():
    global _NC_CACHE
    if _NC_CACHE is None:
        _NC_CACHE = _build_nc()
    return _NC_CACHE


# ---------------------------------------------------------------------------
# Host wrapper
# ---------------------------------------------------------------------------

def kernel(x, W_qkv, W_proj, cos, sin, mask):
    bf = ml_dtypes.bfloat16
    x = np.asarray(x, dtype=np.float32)
    W_qkv = np.asarray(W_qkv, dtype=np.float32)
    W_proj = np.asarray(W_proj, dtype=np.float32)
    cos = np.asarray(cos, dtype=np.float32)
    sin = np.asarray(sin, dtype=np.float32)

    # Permute q/k head dims: interleaved (x1,x2 pairs) -> halves [x1; x2].
    perm = np.concatenate([np.arange(0, HD, 2), np.arange(1, HD, 2)])
    Wq = W_qkv[0:C].reshape(H, HD, C)[:, perm, :].reshape(C, C)
    Wk = W_qkv[C:2 * C].reshape(H, HD, C)[:, perm, :].reshape(C, C)
    Wv = W_qkv[2 * C:3 * C]

    # full-H tiled layouts: [8 pairs, 128 c-part, CC, 128 d]
    wqt_full = np.ascontiguousarray(
        Wq.T.astype(bf).reshape(CC, 128, 8, 128).transpose(2, 1, 0, 3)
    )
    wkt_full = np.ascontiguousarray(
        Wk.T.astype(bf).reshape(CC, 128, 8, 128).transpose(2, 1, 0, 3)
    )
    # V weights in 4-head group slabs: [4 groups, 128 c-part, CC, 256]
    wvg_full = np.ascontiguousarray(
        Wv.T.astype(bf).reshape(CC, 128, 4, 256).transpose(2, 1, 0, 3)
    )
    wpt_full = W_proj.T.astype(bf)   # [1024 rows (channels), 1024]

    cosT = cos.T
    sinT = sin.T
    cosr = np.ascontiguousarray(np.tile(cosT, (4, 1)).astype(bf))
    sinB = np.ascontiguousarray(
        np.tile(np.concatenate([sinT, -sinT], axis=0), (2, 1)).astype(bf)
    )

    in_maps = []
    for c in range(NCORES):
        b, hh = divmod(c, 2)
        xtb = np.ascontiguousarray(x[b].T.astype(bf))
        in_maps.append(
            {
                "xt": xtb,
                "wqt": np.ascontiguousarray(wqt_full[4 * hh:4 * hh + 4]),
                "wkt": np.ascontiguousarray(wkt_full[4 * hh:4 * hh + 4]),
                "wvg": np.ascontiguousarray(wvg_full[2 * hh:2 * hh + 2]),
                "wpt": np.ascontiguousarray(
                    wpt_full[512 * hh:512 * hh + 512]),
                "cosk": cosr,
                "sink": sinB,
            }
        )

    nc = _get_nc()
    trace = bool(int(os.environ.get("BASSK_TRACE", "0")))
    res = bass_utils.run_bass_kernel_spmd(
        nc, in_maps, core_ids=list(range(NCORES)), trace=trace
    )
    if trace:
        kernel.last_exec_time_ns = res.exec_time_ns
        kernel.last_profile = res

    out = np.empty((B, T, C), dtype=np.float32)
    for b in range(B):
        out[b] = (res.results[2 * b]["out"].astype(np.float32)
                  + res.results[2 * b + 1]["out"].astype(np.float32))
    return out
